# revision 1
# baseline (speedup 1.0000x reference)
"""TRN2 Bass kernel for nn_DoubleGSOFTCrossAttnProcessor.

Strategy
--------
The GSOFT block-diagonal orthogonal transforms (Cayley maps of tiny [16,b,b]
parameter blocks) are linear, so they fold into the dense projection weights
on the host:

    q = q_scale * gsoft(gsoft(x, Pq_in) @ Wq.T, Pq_out)
      = x @ [BD(Q(Pq_in)) @ Wq.T @ BD(Q(Pq_out)) @ diag(q_scale)] = x @ Wq_eff

(same for k, v and the output projection; the bias is added on the host after
the device pass). The device kernel is then plain 8-head cross-attention with
effective weights, data-parallel over batch: 8 batch elements -> 8 NeuronCores,
weights replicated, no collectives.

Device kernel (per core, all matmuls in float32r = TF32-like, fp32 PSUM):
  - Q^T = Wq_eff^T @ x^T per 512-seq tile (features on partitions).
  - scores^T[s_k, s_q] per head; softmax without max-subtraction (scores are
    O(5), exp can't overflow fp32): exp on ScalarE, key-sum via ones-matmul,
    reciprocal on VectorE, partition-broadcast via ones-matmul, normalize in
    place. Heads flow through a depth-3 software pipeline, and the previous
    tile's output-projection matmul groups are interleaved between the
    pipeline's dependent links as PE gap fillers.
  - attnout^T = V_h^T @ probs^T, evicted into a feature-permuted layout
    (HEAD_PERM) so every head's 160 features land 128-aligned.
  - out = attnout^T.T @ Wout_eff per 128-row seq chunk, DMA'd out.

HEAD_PERM: head h's first 128 score/value features -> chunk h; its last 32
packed into chunks 8-9 at row 32*(h%4). Applied to Wq/Wk columns, Wv columns
and Wout rows on the host, which makes every matmul operand and PSUM eviction
partition-aligned (the 160-dim head size is otherwise hostile to the
128-partition PE geometry).
"""


import numpy as np
from contextlib import ExitStack

import concourse.bass as bass
import concourse.bass_isa as bass_isa
import concourse.tile as tile
from concourse import bacc, mybir

F32 = mybir.dt.float32
F32R = mybir.dt.float32r

HID, CROSS, NBLK, HEADS = 1280, 768, 16, 8
HEAD_DIM = HID // HEADS               # 160
ATTN_SCALE = HEAD_DIM ** -0.5
SEQ, SKEY = 4096, 77
SKP = 80                              # padded key count (even, f32r requirement)
SQ = 512                              # seq-tile size
NT = SEQ // SQ                        # 8 seq tiles
KH, KC = HID // 128, CROSS // 128     # 10, 6 contraction chunks
XH = KH * SQ // 2                     # xt half-tile free size (2560)
NTILES = [(0, 512), (512, 512), (1024, 256)]  # featout tiles


def _cayley(P):
    P = P.astype(np.float64)
    A = P - np.swapaxes(P, -1, -2)
    I = np.eye(P.shape[-1], dtype=np.float64)
    return np.linalg.solve(I[None] - A, np.broadcast_to(I, A.shape) + A)


def _fold(P_in, W, P_out, scale):
    """W_eff = BD(Q_in) @ W.T @ BD(Q_out) @ diag(scale); W is [out, in]."""
    Qi, Qo = _cayley(P_in), _cayley(P_out)
    WT = W.astype(np.float64).T
    g, b = Qi.shape[0], Qi.shape[1]
    T1 = np.einsum("gij,gjc->gic", Qi, WT.reshape(g, b, -1)).reshape(WT.shape)
    go, bo = Qo.shape[0], Qo.shape[1]
    T2 = np.einsum("rgi,gij->rgj", T1.reshape(-1, go, bo), Qo).reshape(WT.shape)
    return T2 * scale.astype(np.float64)[None, :]


def _head_perm():
    """head h's first 128 features -> chunk h; last 32 -> chunk 8/9 row 32*(h%4)."""
    perm = np.empty(HID, np.int64)
    for h in range(HEADS):
        perm[128 * h : 128 * h + 128] = np.arange(160 * h, 160 * h + 128)
        perm[1024 + 32 * h : 1024 + 32 * h + 32] = np.arange(
            160 * h + 128, 160 * h + 160)
    return perm


HEAD_PERM = _head_perm()


def fold_weights(inputs):
    wq = _fold(inputs["Pq_in"], inputs["Wq"], inputs["Pq_out"], inputs["q_scale"])
    wk = _fold(inputs["Pk_in"], inputs["Wk"], inputs["Pk_out"], inputs["k_scale"])
    wv = _fold(inputs["Pv_in"], inputs["Wv"], inputs["Pv_out"], inputs["v_scale"])
    wo = _fold(inputs["Pout_in"], inputs["Wout"], inputs["Pout_out"],
               inputs["out_scale"])
    wq = wq[:, HEAD_PERM]
    wk = wk[:, HEAD_PERM]
    wv = wv[:, HEAD_PERM]
    wo = wo[HEAD_PERM, :]
    return (wq.astype(np.float32), wk.astype(np.float32),
            wv.astype(np.float32), wo.astype(np.float32))


def _pack_w(W):  # [K*128, M] -> [128, K*M]
    Kc = W.shape[0] // 128
    return np.ascontiguousarray(
        W.reshape(Kc, 128, W.shape[1]).transpose(1, 0, 2).reshape(128, -1))


def make_in_map(x_b, enc_b, wq, wk, wv, wo):
    xt = (x_b.T.reshape(KH, 128, NT, SQ).transpose(2, 1, 0, 3)
          .reshape(NT, 128, 2, XH).transpose(0, 2, 1, 3))
    xt = np.ascontiguousarray(xt)                    # [NT, 2, 128, XH]
    encp = np.zeros((SKP, CROSS), np.float32)
    encp[:SKEY] = enc_b
    enct = _pack_w(np.ascontiguousarray(encp.T))
    return {
        "xt": xt,
        "wq": _pack_w(wq), "wk": _pack_w(wk), "wv": _pack_w(wv), "wo": _pack_w(wo),
        "enct": enct,
        "ones": np.ones((128, SKP), np.float32),
    }


def _head_pieces(h):
    return [(h, 0, 128), (8 + h // 4, 32 * (h % 4), 32)]


def build_nc(loop_reps=1):
    nc = bacc.Bacc("TRN2", target_bir_lowering=False, debug=False)
    xt_d = nc.dram_tensor("xt", [NT, 2, 128, XH], F32R, kind="ExternalInput").ap()
    wq_d = nc.dram_tensor("wq", [128, KH * HID], F32R, kind="ExternalInput").ap()
    wk_d = nc.dram_tensor("wk", [128, KC * HID], F32R, kind="ExternalInput").ap()
    wv_d = nc.dram_tensor("wv", [128, KC * HID], F32R, kind="ExternalInput").ap()
    wo_d = nc.dram_tensor("wo", [128, KH * HID], F32R, kind="ExternalInput").ap()
    enct_d = nc.dram_tensor("enct", [128, KC * SKP], F32R, kind="ExternalInput").ap()
    ones_d = nc.dram_tensor("ones", [128, SKP], F32R, kind="ExternalInput").ap()
    out_d = nc.dram_tensor("out", [SEQ, HID], F32, kind="ExternalOutput").ap()

    with tile.TileContext(nc) as tc:
        with ExitStack() as ctx:
            ctx.enter_context(nc.allow_low_precision(
                "f32r matmul inputs; accumulation stays f32 in PSUM"))
            const = ctx.enter_context(tc.tile_pool(name="const", bufs=1))
            # order matters: wq + first xt halves first so B(0) starts early
            wq_t = const.tile([128, KH * HID], F32R, name="wq_t")
            nc.sync.dma_start(wq_t[:], wq_d)
            ones_t = const.tile([128, SKP], F32R, name="ones_t")
            nc.sync.dma_start(ones_t[:], ones_d)
            kt_t = const.tile([128, KH * SKP], F32R, name="kt_t")
            v_t = const.tile([128, HID], F32R, name="v_t")
            wo_t = const.tile([128, KH * HID], F32R, name="wo_t")

            xt_pool = ctx.enter_context(tc.tile_pool(name="xt", bufs=2))
            qt_pool = ctx.enter_context(tc.tile_pool(name="qt", bufs=1))
            psum_mm = ctx.enter_context(
                tc.tile_pool(name="psum_mm", bufs=2, space="PSUM"))

            if loop_reps > 1:
                # hint_engines: the ~2900-inst body exceeds IRAM blocks, so
                # prefetch the back-edge target (else ~4us I$ miss/iteration
                # inflates the measured per-pass slope)
                ctx.enter_context(tc.For_i(
                    0, loop_reps, 1,
                    hint_engines=(mybir.EngineType.PE, mybir.EngineType.DVE,
                                  mybir.EngineType.Activation,
                                  mybir.EngineType.SP, mybir.EngineType.Pool)))

            qt_tiles = {}

            def phase_B(t):
                xh = []
                for hf in range(2):
                    xx = xt_pool.tile([128, XH], F32R, tag="xt", name=f"xt{t}_{hf}")
                    nc.sync.dma_start(xx[:], xt_d[t, hf])
                    xh.append(xx)
                qt_t = qt_pool.tile([128, KH * SQ], F32R, tag="qt", name=f"qt{t}")
                for m in range(KH):
                    pq = psum_mm.tile([128, SQ], F32, tag="mm", name=f"pq{t}_{m}")
                    for k in range(KH):
                        nc.tensor.matmul(
                            pq[:],
                            wq_t[:, k * HID + m * 128 : k * HID + (m + 1) * 128],
                            xh[k // 5][:, (k % 5) * SQ : (k % 5 + 1) * SQ],
                            start=(k == 0), stop=(k == KH - 1),
                        )
                    nc.vector.tensor_copy(qt_t[:, m * SQ : (m + 1) * SQ], pq[:])
                qt_tiles[t] = qt_t

            phase_B(0)

            # ------- setup: KT = Wk_eff^T @ enc^T, V = enc @ Wv_eff (after B0)
            with tc.tile_pool(name="setup_e", bufs=1) as setup_e, \
                 tc.tile_pool(name="psum_setup", bufs=2, space="PSUM") as psum_s:
                enct_t = setup_e.tile([128, KC * SKP], F32R, name="enct_t")
                nc.sync.dma_start(enct_t[:], enct_d)
                with tc.tile_pool(name="setup_k", bufs=1) as setup_k:
                    wk_t = setup_k.tile([128, KC * HID], F32R, name="wk_t")
                    nc.sync.dma_start(wk_t[:], wk_d)
                    for m in range(KH):
                        pk = psum_s.tile([128, SKP], F32, tag="pk", name=f"pk{m}")
                        for k in range(KC):
                            nc.tensor.matmul(
                                pk[:],
                                wk_t[:, k * HID + m * 128 : k * HID + (m + 1) * 128],
                                enct_t[:, k * SKP : (k + 1) * SKP],
                                start=(k == 0), stop=(k == KC - 1),
                            )
                        nc.vector.tensor_copy(kt_t[:, m * SKP : (m + 1) * SKP], pk[:])
                with tc.tile_pool(name="setup_v", bufs=1) as setup_v:
                    wv_t = setup_v.tile([128, KC * HID], F32R, name="wv_t")
                    nc.sync.dma_start(wv_t[:], wv_d)
                    for (n_off, n_sz) in NTILES:
                        pv = psum_s.tile([SKEY, n_sz], F32, tag="pk", name=f"pv{n_off}")
                        for k in range(KC):
                            nc.tensor.matmul(
                                pv[:],
                                enct_t[:, k * SKP : k * SKP + SKEY],
                                wv_t[:, k * HID + n_off : k * HID + n_off + n_sz],
                                start=(k == 0), stop=(k == KC - 1),
                            )
                        nc.vector.tensor_copy(v_t[0:SKEY, n_off : n_off + n_sz], pv[:])

            # wo arrives while B(0)/setup computes
            nc.sync.dma_start(wo_t[:], wo_d)

            # ------- main pools (reuse the setup space)
            ot_pool = ctx.enter_context(tc.tile_pool(name="ot", bufs=2))
            exp_pool = ctx.enter_context(tc.tile_pool(name="exp", bufs=3))
            rc_pool = ctx.enter_context(tc.tile_pool(name="rc", bufs=2))
            out_pool = ctx.enter_context(tc.tile_pool(name="outsb", bufs=1))
            psum_at = ctx.enter_context(
                tc.tile_pool(name="psum_at", bufs=4, space="PSUM"))
            psum_av = ctx.enter_context(
                tc.tile_pool(name="psum_av", bufs=2, space="PSUM"))

            ot_tiles = {}

            def d_group_makers(t):
                """D-phase of tile t as a list of closures (12 matmul groups,
                store after each 128-row chunk's last group)."""
                ot_t = ot_tiles.pop(t)
                sbs = {}
                makers = []

                def mk(j, n_off, n_sz):
                    def run():
                        if j not in sbs:
                            sbs[j] = out_pool.tile([128, HID], F32, tag="osb",
                                                   name=f"ob{t}_{j}")
                        po = psum_mm.tile([128, n_sz], F32, tag="mm",
                                          name=f"po{t}_{j}_{n_off}")
                        for c in range(KH):
                            nc.tensor.matmul(
                                po[:],
                                ot_t[:, c * SQ + j * 128 : c * SQ + (j + 1) * 128],
                                wo_t[:, c * HID + n_off : c * HID + n_off + n_sz],
                                start=(c == 0), stop=(c == KH - 1),
                            )
                        nc.vector.tensor_copy(sbs[j][:, n_off : n_off + n_sz], po[:])
                        if n_off == NTILES[-1][0]:
                            nc.sync.dma_start(
                                out_d[t * SQ + j * 128 : t * SQ + (j + 1) * 128, :],
                                sbs[j][:],
                            )
                    return run

                for j in range(SQ // 128):
                    for (n_off, n_sz) in NTILES:
                        makers.append(mk(j, n_off, n_sz))
                return makers

            def phase_C(t, fillers):
                """Attention with depth-3 head pipeline; `fillers` (D-groups of
                t-1) emitted between dependent links as PE gap fillers."""
                qt_t = qt_tiles.pop(t)
                ot_t = ot_pool.tile([128, KH * SQ], F32R, tag="ot", name=f"ot{t}")
                exp_tiles, rcs = {}, {}

                def fill(n=1):
                    for _ in range(n):
                        if fillers:
                            fillers.pop(0)()

                def stage1(h):  # scoresT + exp
                    sc = psum_at.tile([SKP, SQ], F32, tag="attn", name=f"sc{t}_{h}")
                    for i, (c, o, L) in enumerate(_head_pieces(h)):
                        nc.tensor.matmul(
                            sc[:],
                            kt_t[o : o + L, c * SKP : (c + 1) * SKP],
                            qt_t[o : o + L, c * SQ : (c + 1) * SQ],
                            start=(i == 0), stop=(i == 1),
                            tile_position=(o, 0),
                        )
                    exp_h = exp_pool.tile([SKEY, SQ], F32R, tag="exp", name=f"ex{t}_{h}")
                    nc.scalar.activation(
                        exp_h[:], sc[0:SKEY, :],
                        mybir.ActivationFunctionType.Exp, scale=ATTN_SCALE,
                    )
                    exp_tiles[h] = exp_h

                def stage2(h):  # key-sum + reciprocal
                    sm = psum_at.tile([1, SQ], F32, tag="attn", name=f"sm{t}_{h}")
                    nc.tensor.matmul(sm[:], ones_t[0:SKEY, 0:1], exp_tiles[h][:],
                                     start=True, stop=True)
                    rc = rc_pool.tile([1, SQ], F32R, tag="rc", name=f"rc{t}_{h}")
                    nc.vector.reciprocal(rc[:], sm[:])
                    rcs[h] = rc

                def stage34(h):  # bcast + normalize, fill, then attnout
                    bc = psum_at.tile([SKEY, SQ], F32, tag="attn", name=f"bc{t}_{h}")
                    nc.tensor.matmul(bc[:], ones_t[0:1, 0:SKEY], rcs.pop(h)[:],
                                     start=True, stop=True)
                    nc.vector.tensor_tensor(exp_tiles[h][:], exp_tiles[h][:],
                                            bc[:], mybir.AluOpType.mult)
                    fill()  # PE gap while DVE normalizes
                    exp_h = exp_tiles.pop(h)
                    for (c, o, L, pname) in [(h, 0, 128, "pa"),
                                             (8 + h // 4, 32 * (h % 4), 32, "pb")]:
                        pos = c * 128 + o
                        pa = psum_av.tile([L, SQ], F32, tag="att",
                                          name=f"{pname}{t}_{h}")
                        nc.tensor.matmul(pa[:], v_t[0:SKEY, pos : pos + L],
                                         exp_h[:], start=True, stop=True)
                        nc.vector.tensor_copy(
                            ot_t[o : o + L, c * SQ : (c + 1) * SQ], pa[:])

                for s in range(HEADS + 2):
                    if s < HEADS:
                        stage1(s)
                    fill()
                    if 0 <= s - 1 < HEADS:
                        stage2(s - 1)
                    fill()
                    if 0 <= s - 2 < HEADS:
                        stage34(s - 2)
                while fillers:
                    fillers.pop(0)()
                ot_tiles[t] = ot_t

            for t in range(NT):
                if t > 0:
                    phase_B(t)
                fillers = d_group_makers(t - 1) if t > 0 else []
                phase_C(t, fillers)
            for run in d_group_makers(NT - 1):
                run()

    nc.finalize()
    return nc


from concourse.bass_utils import run_bass_kernel_spmd

_NC_CACHE = {}


def _get_nc(loop_reps=1):
    if loop_reps not in _NC_CACHE:
        _NC_CACHE[loop_reps] = build_nc(loop_reps)
    return _NC_CACHE[loop_reps]


def kernel(**inputs):
    inputs = {k: np.asarray(v) for k, v in inputs.items()}
    wq, wk, wv, wo = fold_weights(inputs)
    x = inputs["hidden_states"].astype(np.float32, copy=False)
    enc = inputs["encoder_hidden_states"].astype(np.float32, copy=False)
    B = x.shape[0]
    in_maps = [make_in_map(x[b], enc[b], wq, wk, wv, wo) for b in range(B)]
    nc = _get_nc()
    res = run_bass_kernel_spmd(nc, in_maps, list(range(B)))
    bout = inputs["bout"].astype(np.float32, copy=False)
    return np.stack([res.results[b]["out"] + bout[None, :] for b in range(B)])



# revision 2
# speedup vs baseline: 1.2726x; 1.2726x over previous
"""TRN2 Bass kernel for nn_DoubleGSOFTCrossAttnProcessor.

Strategy
--------
The GSOFT block-diagonal orthogonal transforms (Cayley maps of tiny [16,b,b]
parameter blocks) are linear, so they fold into the dense projection weights
on the host:

    q = q_scale * gsoft(gsoft(x, Pq_in) @ Wq.T, Pq_out)
      = x @ [BD(Q(Pq_in)) @ Wq.T @ BD(Q(Pq_out)) @ diag(q_scale)] = x @ Wq_eff

(same for k, v and the output projection; the bias is added on the host after
the device pass). The device kernel is then plain 8-head cross-attention with
effective weights, data-parallel over batch: 8 batch elements -> 8 NeuronCores,
weights replicated, no collectives.

Device kernel (per core). Everything is bf16 (activations, weights, SBUF
intermediates) with fp32 PSUM accumulation; bf16 keeps the big matmuls at
1 PE-cycle/row, halves HBM traffic and SBUF footprint, and enables the DVE
2x 16-bit path. Verified numerically on the host: ~6e-3 max rel err.

  - B(t): Q^T = Wq_eff^T @ x^T per 512-seq tile (features on partitions).
  - C(t): per-head attention softmax, no max-subtraction (scores are O(5)).
    Depth-4 head pipeline:
      s1 scores^T (PE, 2 matmuls via HEAD_PERM) -> exp (Act, PSUM->bf16)
      s2 key-sum via ones-matmul (PE) -> reciprocal (DVE)
      s3 partition_broadcast of 1/sum (Pool/GpSimd, SBUF only)
         -> probs normalize (DVE 16-bit tensor_tensor)
      s4 attnout^T = V_h^T @ probs^T (PE) -> evict to ot (Act/DVE)
  - D(t): out = attnout^T.T @ Wout_eff per 128-row seq chunk, DMA'd out bf16.

PE is the bottleneck engine (~410us of matmul rows), so B/D matmul work is
sliced into 3-4-matmul chunks and paced evenly through C(t)'s dependency
links as gap fillers: C(t) consumes D(t-1) first (its ot is ready), then
B(t+1) (whose xt DMA is issued at C(t) entry, ~25us before first use).
Eviction copies alternate Act/DVE; Pool only ever touches SBUF (no PSUM
port on GpSimd).

HEAD_PERM: head h's first 128 score/value features -> chunk h; its last 32
packed into chunks 8-9 at row 32*(h%4). Applied to Wq/Wk columns, Wv columns
and Wout rows on the host, which makes every matmul operand and PSUM eviction
partition-aligned (the 160-dim head size is otherwise hostile to the
128-partition PE geometry).
"""


import numpy as np
from contextlib import ExitStack

import ml_dtypes

import concourse.bass as bass
import concourse.bass_isa as bass_isa
import concourse.tile as tile
from concourse import bacc, mybir

F32 = mybir.dt.float32
BF16 = mybir.dt.bfloat16
NPBF16 = ml_dtypes.bfloat16

HID, CROSS, NBLK, HEADS = 1280, 768, 16, 8
HEAD_DIM = HID // HEADS               # 160
ATTN_SCALE = HEAD_DIM ** -0.5
SEQ, SKEY = 4096, 77
SKP = 80                              # padded key count
SQ = 512                              # seq-tile size
NT = SEQ // SQ                        # 8 seq tiles
KH, KC = HID // 128, CROSS // 128     # 10, 6 contraction chunks
XH = KH * SQ // 2                     # xt half-tile free size (2560)
NTILES = [(0, 512), (512, 512), (1024, 256)]  # featout tiles
CHUNKS = [(0, 4), (4, 3), (7, 3)]     # matmul-group split for PE gap filling


def _cayley(P):
    P = P.astype(np.float64)
    A = P - np.swapaxes(P, -1, -2)
    I = np.eye(P.shape[-1], dtype=np.float64)
    return np.linalg.solve(I[None] - A, np.broadcast_to(I, A.shape) + A)


def _fold(P_in, W, P_out, scale):
    """W_eff = BD(Q_in) @ W.T @ BD(Q_out) @ diag(scale); W is [out, in]."""
    Qi, Qo = _cayley(P_in), _cayley(P_out)
    WT = W.astype(np.float64).T
    g, b = Qi.shape[0], Qi.shape[1]
    T1 = np.einsum("gij,gjc->gic", Qi, WT.reshape(g, b, -1)).reshape(WT.shape)
    go, bo = Qo.shape[0], Qo.shape[1]
    T2 = np.einsum("rgi,gij->rgj", T1.reshape(-1, go, bo), Qo).reshape(WT.shape)
    return T2 * scale.astype(np.float64)[None, :]


def _head_perm():
    """head h's first 128 features -> chunk h; last 32 -> chunk 8/9 row 32*(h%4)."""
    perm = np.empty(HID, np.int64)
    for h in range(HEADS):
        perm[128 * h : 128 * h + 128] = np.arange(160 * h, 160 * h + 128)
        perm[1024 + 32 * h : 1024 + 32 * h + 32] = np.arange(
            160 * h + 128, 160 * h + 160)
    return perm


HEAD_PERM = _head_perm()


def fold_weights(inputs):
    wq = _fold(inputs["Pq_in"], inputs["Wq"], inputs["Pq_out"], inputs["q_scale"])
    wk = _fold(inputs["Pk_in"], inputs["Wk"], inputs["Pk_out"], inputs["k_scale"])
    wv = _fold(inputs["Pv_in"], inputs["Wv"], inputs["Pv_out"], inputs["v_scale"])
    wo = _fold(inputs["Pout_in"], inputs["Wout"], inputs["Pout_out"],
               inputs["out_scale"])
    wq = wq[:, HEAD_PERM]
    wk = wk[:, HEAD_PERM]
    wv = wv[:, HEAD_PERM]
    wo = wo[HEAD_PERM, :]
    return (wq.astype(np.float32), wk.astype(np.float32),
            wv.astype(np.float32), wo.astype(np.float32))


def _pack_w(W):  # [K*128, M] -> [128, K*M]
    Kc = W.shape[0] // 128
    return np.ascontiguousarray(
        W.reshape(Kc, 128, W.shape[1]).transpose(1, 0, 2).reshape(128, -1))


def make_in_map(x_b, enc_b, wq, wk, wv, wo):
    xt = (x_b.T.reshape(KH, 128, NT, SQ).transpose(2, 1, 0, 3)
          .reshape(NT, 128, 2, XH).transpose(0, 2, 1, 3))
    xt = np.ascontiguousarray(xt).astype(NPBF16)     # [NT, 2, 128, XH]
    encp = np.zeros((SKP, CROSS), np.float32)
    encp[:SKEY] = enc_b
    enct = _pack_w(np.ascontiguousarray(encp.T)).astype(NPBF16)
    return {
        "xt": xt,
        "wq": _pack_w(wq).astype(NPBF16), "wk": _pack_w(wk).astype(NPBF16),
        "wv": _pack_w(wv).astype(NPBF16), "wo": _pack_w(wo).astype(NPBF16),
        "enct": enct,
        "ones": np.ones((128, SKP), NPBF16),
    }


def _head_pieces(h):
    return [(h, 0, 128), (8 + h // 4, 32 * (h % 4), 32)]


def build_nc(loop_reps=1):
    nc = bacc.Bacc("TRN2", target_bir_lowering=False, debug=False)
    xt_d = nc.dram_tensor("xt", [NT, 2, 128, XH], BF16, kind="ExternalInput").ap()
    wq_d = nc.dram_tensor("wq", [128, KH * HID], BF16, kind="ExternalInput").ap()
    wk_d = nc.dram_tensor("wk", [128, KC * HID], BF16, kind="ExternalInput").ap()
    wv_d = nc.dram_tensor("wv", [128, KC * HID], BF16, kind="ExternalInput").ap()
    wo_d = nc.dram_tensor("wo", [128, KH * HID], BF16, kind="ExternalInput").ap()
    enct_d = nc.dram_tensor("enct", [128, KC * SKP], BF16, kind="ExternalInput").ap()
    ones_d = nc.dram_tensor("ones", [128, SKP], BF16, kind="ExternalInput").ap()
    out_d = nc.dram_tensor("out", [SEQ, HID], BF16, kind="ExternalOutput").ap()

    with tile.TileContext(nc) as tc:
        with ExitStack() as ctx:
            ctx.enter_context(nc.allow_low_precision(
                "bf16 matmul inputs and SBUF intermediates; f32 PSUM accum"))
            const = ctx.enter_context(tc.tile_pool(name="const", bufs=1))
            # order matters: wq + first xt halves first so B(0) starts early
            wq_t = const.tile([128, KH * HID], BF16, name="wq_t")
            nc.sync.dma_start(wq_t[:], wq_d)
            ones_t = const.tile([128, SKP], BF16, name="ones_t")
            nc.sync.dma_start(ones_t[:], ones_d)
            kt_t = const.tile([128, KH * SKP], BF16, name="kt_t")
            v_t = const.tile([128, HID], BF16, name="v_t")
            wo_t = const.tile([128, KH * HID], BF16, name="wo_t")

            xt_pool = ctx.enter_context(tc.tile_pool(name="xt", bufs=4))
            qt_pool = ctx.enter_context(tc.tile_pool(name="qt", bufs=2))
            psum_mm = ctx.enter_context(
                tc.tile_pool(name="psum_mm", bufs=2, space="PSUM"))

            if loop_reps > 1:
                # hint_engines: the ~2700-inst body exceeds IRAM blocks, so
                # prefetch the back-edge target (else ~4us I$ miss/iteration
                # inflates the measured per-pass slope)
                ctx.enter_context(tc.For_i(
                    0, loop_reps, 1,
                    hint_engines=(mybir.EngineType.PE, mybir.EngineType.DVE,
                                  mybir.EngineType.Activation,
                                  mybir.EngineType.SP, mybir.EngineType.Pool)))

            qt_tiles = {}
            ev_ctr = [0]

            def ev_copy(dst, src):
                """PSUM->SBUF eviction, alternating Act / DVE."""
                ev_ctr[0] += 1
                if ev_ctr[0] % 2:
                    nc.scalar.activation(dst, src,
                                         mybir.ActivationFunctionType.Copy)
                else:
                    nc.vector.tensor_copy(dst, src)

            def b_work(t):
                """Phase B of tile t as (dma_closure, chunk list).  Each chunk
                is 3-4 matmuls; the last chunk of each m-group evicts."""
                state = {}

                def start_dma():
                    xh = []
                    for hf in range(2):
                        xx = xt_pool.tile([128, XH], BF16, tag="xt",
                                          name=f"xt{t}_{hf}")
                        nc.sync.dma_start(xx[:], xt_d[t, hf])
                        xh.append(xx)
                    state["xh"] = xh
                    qt_tiles[t] = qt_pool.tile([128, KH * SQ], BF16, tag="qt",
                                               name=f"qt{t}")

                def mk(m, k0, kn, last):
                    def run():
                        xh = state["xh"]
                        if k0 == 0:
                            state["pq"] = psum_mm.tile(
                                [128, SQ], F32, tag="mm", name=f"pq{t}_{m}")
                        pq = state["pq"]
                        for k in range(k0, k0 + kn):
                            nc.tensor.matmul(
                                pq[:],
                                wq_t[:, k * HID + m * 128 : k * HID + (m + 1) * 128],
                                xh[k // 5][:, (k % 5) * SQ : (k % 5 + 1) * SQ],
                                start=(k == 0), stop=(k == KH - 1),
                            )
                        if last:
                            ev_copy(qt_tiles[t][:, m * SQ : (m + 1) * SQ], pq[:])
                    return run

                chunks = []
                for m in range(KH):
                    for ci, (k0, kn) in enumerate(CHUNKS):
                        chunks.append(mk(m, k0, kn, ci == len(CHUNKS) - 1))
                return start_dma, chunks

            # ------- B(0) standalone
            b0_dma, b0_chunks = b_work(0)
            b0_dma()
            for c in b0_chunks:
                c()

            # ------- setup: KT = Wk_eff^T @ enc^T, V = enc @ Wv_eff (after B0)
            with tc.tile_pool(name="setup_e", bufs=1) as setup_e, \
                 tc.tile_pool(name="psum_setup", bufs=2, space="PSUM") as psum_s:
                enct_t = setup_e.tile([128, KC * SKP], BF16, name="enct_t")
                nc.sync.dma_start(enct_t[:], enct_d)
                with tc.tile_pool(name="setup_k", bufs=1) as setup_k:
                    wk_t = setup_k.tile([128, KC * HID], BF16, name="wk_t")
                    nc.sync.dma_start(wk_t[:], wk_d)
                    for m in range(KH):
                        pk = psum_s.tile([128, SKP], F32, tag="pk", name=f"pk{m}")
                        for k in range(KC):
                            nc.tensor.matmul(
                                pk[:],
                                wk_t[:, k * HID + m * 128 : k * HID + (m + 1) * 128],
                                enct_t[:, k * SKP : (k + 1) * SKP],
                                start=(k == 0), stop=(k == KC - 1),
                            )
                        nc.vector.tensor_copy(kt_t[:, m * SKP : (m + 1) * SKP], pk[:])
                with tc.tile_pool(name="setup_v", bufs=1) as setup_v:
                    wv_t = setup_v.tile([128, KC * HID], BF16, name="wv_t")
                    nc.sync.dma_start(wv_t[:], wv_d)
                    for (n_off, n_sz) in NTILES:
                        pv = psum_s.tile([SKEY, n_sz], F32, tag="pk", name=f"pv{n_off}")
                        for k in range(KC):
                            nc.tensor.matmul(
                                pv[:],
                                enct_t[:, k * SKP : k * SKP + SKEY],
                                wv_t[:, k * HID + n_off : k * HID + n_off + n_sz],
                                start=(k == 0), stop=(k == KC - 1),
                            )
                        nc.vector.tensor_copy(v_t[0:SKEY, n_off : n_off + n_sz], pv[:])

            # wo arrives while B(0)/setup computes
            nc.sync.dma_start(wo_t[:], wo_d)

            # ------- main pools (reuse the setup space)
            ot_pool = ctx.enter_context(tc.tile_pool(name="ot", bufs=2))
            exp_pool = ctx.enter_context(tc.tile_pool(name="exp", bufs=4))
            rc_pool = ctx.enter_context(tc.tile_pool(name="rc", bufs=2))
            bc_pool = ctx.enter_context(tc.tile_pool(name="bc", bufs=2))
            out_pool = ctx.enter_context(tc.tile_pool(name="outsb", bufs=2))
            psum_at = ctx.enter_context(
                tc.tile_pool(name="psum_at", bufs=4, space="PSUM"))
            psum_av = ctx.enter_context(
                tc.tile_pool(name="psum_av", bufs=2, space="PSUM"))

            ot_tiles = {}

            def d_work(t):
                """Phase D of tile t as chunk list (12 matmul groups split into
                3-4-matmul chunks; evict + DMA ride with each group's last)."""
                ot_t = ot_tiles.pop(t)
                sbs = {}
                state = {}

                def mk(j, n_off, n_sz, c0, cn, last):
                    def run():
                        if j not in sbs:
                            sbs[j] = out_pool.tile([128, HID], BF16, tag="osb",
                                                   name=f"ob{t}_{j}")
                        if c0 == 0:
                            state["po"] = psum_mm.tile(
                                [128, n_sz], F32, tag="mm",
                                name=f"po{t}_{j}_{n_off}")
                        po = state["po"]
                        for c in range(c0, c0 + cn):
                            nc.tensor.matmul(
                                po[:],
                                ot_t[:, c * SQ + j * 128 : c * SQ + (j + 1) * 128],
                                wo_t[:, c * HID + n_off : c * HID + n_off + n_sz],
                                start=(c == 0), stop=(c == KH - 1),
                            )
                        if last:
                            ev_copy(sbs[j][:, n_off : n_off + n_sz], po[:])
                            if n_off == NTILES[-1][0]:
                                nc.sync.dma_start(
                                    out_d[t * SQ + j * 128 : t * SQ + (j + 1) * 128, :],
                                    sbs[j][:],
                                )
                    return run

                chunks = []
                for j in range(SQ // 128):
                    for (n_off, n_sz) in NTILES:
                        for ci, (c0, cn) in enumerate(CHUNKS):
                            chunks.append(mk(j, n_off, n_sz, c0, cn,
                                             ci == len(CHUNKS) - 1))
                return chunks

            def phase_C(t, fillers, next_b_dma):
                """Attention with depth-4 head pipeline; `fillers` (D(t-1) then
                B(t+1) chunks) paced evenly through the dependency links."""
                qt_t = qt_tiles.pop(t)
                ot_t = ot_pool.tile([128, KH * SQ], BF16, tag="ot", name=f"ot{t}")
                exp_tiles, rcs = {}, {}
                if next_b_dma is not None:
                    next_b_dma()   # xt(t+1) DMA in flight ~25us before use

                n_points = 4 * (HEADS + 3)
                quota = len(fillers) / n_points
                acc = [0.0]

                def fill():
                    acc[0] += quota
                    while acc[0] >= 1.0 and fillers:
                        fillers.pop(0)()
                        acc[0] -= 1.0

                def stage1(h):  # scoresT + exp
                    sc = psum_at.tile([SKP, SQ], F32, tag="attn", name=f"sc{t}_{h}")
                    for i, (c, o, L) in enumerate(_head_pieces(h)):
                        nc.tensor.matmul(
                            sc[:],
                            kt_t[o : o + L, c * SKP : (c + 1) * SKP],
                            qt_t[o : o + L, c * SQ : (c + 1) * SQ],
                            start=(i == 0), stop=(i == 1),
                            tile_position=(o, 0),
                        )
                    exp_h = exp_pool.tile([SKEY, SQ], BF16, tag="exp",
                                          name=f"ex{t}_{h}")
                    nc.scalar.activation(
                        exp_h[:], sc[0:SKEY, :],
                        mybir.ActivationFunctionType.Exp, scale=ATTN_SCALE,
                    )
                    exp_tiles[h] = exp_h

                def stage2(h):  # key-sum + reciprocal
                    sm = psum_at.tile([1, SQ], F32, tag="attn", name=f"sm{t}_{h}")
                    nc.tensor.matmul(sm[:], ones_t[0:SKEY, 0:1], exp_tiles[h][:],
                                     start=True, stop=True)
                    rc = rc_pool.tile([1, SQ], BF16, tag="rc", name=f"rc{t}_{h}")
                    nc.vector.reciprocal(rc[:], sm[:])
                    rcs[h] = rc

                def stage3(h):  # partition-broadcast (Pool) + normalize (DVE)
                    bc = bc_pool.tile([SKEY, SQ], BF16, tag="bc", name=f"bc{t}_{h}")
                    nc.gpsimd.partition_broadcast(bc[:], rcs.pop(h)[:])
                    nc.vector.tensor_tensor(exp_tiles[h][:], exp_tiles[h][:],
                                            bc[:], mybir.AluOpType.mult)

                def stage4(h):  # attnout + evict
                    exp_h = exp_tiles.pop(h)
                    for (c, o, L, pname) in [(h, 0, 128, "pa"),
                                             (8 + h // 4, 32 * (h % 4), 32, "pb")]:
                        pos = c * 128 + o
                        pa = psum_av.tile([L, SQ], F32, tag="att",
                                          name=f"{pname}{t}_{h}")
                        nc.tensor.matmul(pa[:], v_t[0:SKEY, pos : pos + L],
                                         exp_h[:], start=True, stop=True)
                        ev_copy(ot_t[o : o + L, c * SQ : (c + 1) * SQ], pa[:])

                for s in range(HEADS + 3):
                    if s < HEADS:
                        stage1(s)
                    fill()
                    if 0 <= s - 1 < HEADS:
                        stage2(s - 1)
                    fill()
                    if 0 <= s - 2 < HEADS:
                        stage3(s - 2)
                    fill()
                    if 0 <= s - 3 < HEADS:
                        stage4(s - 3)
                    fill()
                while fillers:
                    fillers.pop(0)()
                ot_tiles[t] = ot_t

            b_next = {}
            for t in range(NT):
                fillers = d_work(t - 1) if t > 0 else []
                if t + 1 < NT:
                    dma_c, chunks = b_work(t + 1)
                    fillers = fillers + chunks
                else:
                    dma_c = None
                phase_C(t, fillers, dma_c)
            for run in d_work(NT - 1):
                run()

    nc.finalize()
    return nc


from concourse.bass_utils import run_bass_kernel_spmd

_NC_CACHE = {}


def _get_nc(loop_reps=1):
    if loop_reps not in _NC_CACHE:
        _NC_CACHE[loop_reps] = build_nc(loop_reps)
    return _NC_CACHE[loop_reps]


def kernel(**inputs):
    inputs = {k: np.asarray(v) for k, v in inputs.items()}
    wq, wk, wv, wo = fold_weights(inputs)
    x = inputs["hidden_states"].astype(np.float32, copy=False)
    enc = inputs["encoder_hidden_states"].astype(np.float32, copy=False)
    B = x.shape[0]
    in_maps = [make_in_map(x[b], enc[b], wq, wk, wv, wo) for b in range(B)]
    nc = _get_nc()
    res = run_bass_kernel_spmd(nc, in_maps, list(range(B)))
    bout = inputs["bout"].astype(np.float32, copy=False)
    return np.stack([
        np.asarray(res.results[b]["out"]).astype(np.float32) + bout[None, :]
        for b in range(B)
    ])


# revision 23
# speedup vs baseline: 2.0491x; 1.6101x over previous
"""TRN2 Bass kernel for nn_DoubleGSOFTCrossAttnProcessor.

Strategy
--------
The GSOFT block-diagonal orthogonal transforms (Cayley maps of tiny [16,b,b]
parameter blocks) are linear, so they fold into the dense projection weights
on the host:

    q = q_scale * gsoft(gsoft(x, Pq_in) @ Wq.T, Pq_out) = x @ Wq_eff

(same for k, v, out; bias added on the host).  Data-parallel over batch:
8 batch elements -> 8 NeuronCores, weights replicated, no collectives.

Low-rank fusion: with only 77 keys per head, attention is rank-77 while the
Q/attnout tensors are 160-dim per head, so materializing them wastes FLOPs.
Fold the key side into the Q projection and the value side into the out
projection (computed once per pass on device, bf16):

    M_h = Wq_eff[:, h] @ K_h^T           [1280, 77]   scores_h = x @ M_h
    N_h = V_h @ Wout_eff[h, :]           [77, 1280]   out     = sum_h p_h @ N_h

Total matmul work drops from 14.4 GFLOP to 6.5 GFLOP per core.  The 8x77
key axis is packed into 616 rows (5 chunks of 128 partitions), so both big
GEMM passes run near-full PE occupancy:

  front(t): scores^T chunks [128kp, 512q] = M^T-chunks @ x^T  (PE, 10-chunk
            accumulate) -> exp (Act, no max-subtraction; scores are O(5))
            -> per-head key-sums via one indicator-matrix matmul per chunk
            (PE, accumulated [8, 512]).
  norm(t):  reciprocal (DVE) -> per-head partition_broadcast into packed
            [128, 512] scale tiles (Pool; GpSimd has no PSUM port) ->
            probs = exp * scale (DVE 16-bit tensor_tensor).
  G(t):     out rows = sum_p probs_p^T-chunk @ N-chunk (PE, 5-chunk
            accumulate) -> evict bf16 (Act/DVE alternating) -> DMA.

Everything is bf16 with fp32 PSUM accumulation (verified ~6e-3 max rel err
on the host).  G(t-1)'s matmul groups are sliced into 2-3-matmul chunks and
paced through front(t)/norm(t)'s dependency links as PE gap fillers, so PE
stays ~97% busy in steady state.

Packing: HEAD_PERM puts head h's first 128 q/k features in chunk h and its
last 32 in chunks 8-9 at row 32*(h%4), making every precompute operand
partition-aligned.  Packed-key pad rows (616..639) are zeroed in M (so
exp(0)=1) and in N (so they contribute 0 to the output).
"""


import numpy as np
from contextlib import ExitStack

import ml_dtypes

import concourse.bass as bass
import concourse.bass_isa as bass_isa
import concourse.tile as tile
from concourse import bacc, mybir

F32 = mybir.dt.float32
BF16 = mybir.dt.bfloat16
NPBF16 = ml_dtypes.bfloat16

HID, CROSS, NBLK, HEADS = 1280, 768, 16, 8
HEAD_DIM = HID // HEADS               # 160
ATTN_SCALE = HEAD_DIM ** -0.5
SEQ, SKEY = 4096, 77
SKP = 80                              # padded per-head key count in kt/vt
SQ = 512                              # seq-tile size
NT = SEQ // SQ                        # 8 seq tiles
KH, KC = HID // 128, CROSS // 128     # 10, 6 contraction chunks
XH = KH * SQ // 2                     # xt half-tile free size (2560)
NTILES = [(0, 512), (512, 512), (1024, 256)]  # featout tiles
KP = HEADS * SKEY                     # 616 packed key rows
NP = (KP + 127) // 128                # 5 packed chunks
KPP = NP * 128                        # 640 (incl. pad rows)


def _cayley(P):
    P = P.astype(np.float64)
    A = P - np.swapaxes(P, -1, -2)
    I = np.eye(P.shape[-1], dtype=np.float64)
    return np.linalg.solve(I[None] - A, np.broadcast_to(I, A.shape) + A)


def _fold(P_in, W, P_out, scale):
    """W_eff = BD(Q_in) @ W.T @ BD(Q_out) @ diag(scale); W is [out, in]."""
    Qi, Qo = _cayley(P_in), _cayley(P_out)
    WT = W.astype(np.float64).T
    g, b = Qi.shape[0], Qi.shape[1]
    T1 = np.einsum("gij,gjc->gic", Qi, WT.reshape(g, b, -1)).reshape(WT.shape)
    go, bo = Qo.shape[0], Qo.shape[1]
    T2 = np.einsum("rgi,gij->rgj", T1.reshape(-1, go, bo), Qo).reshape(WT.shape)
    return T2 * scale.astype(np.float64)[None, :]


def _head_perm():
    """head h's first 128 features -> chunk h; last 32 -> chunk 8/9 row 32*(h%4)."""
    perm = np.empty(HID, np.int64)
    for h in range(HEADS):
        perm[128 * h : 128 * h + 128] = np.arange(160 * h, 160 * h + 128)
        perm[1024 + 32 * h : 1024 + 32 * h + 32] = np.arange(
            160 * h + 128, 160 * h + 160)
    return perm


HEAD_PERM = _head_perm()


def _head_pieces(h):
    """(chunk c, row offset o, length L) of head h's dims in PERM space."""
    return [(h, 0, 128), (8 + h // 4, 32 * (h % 4), 32)]


def _sumind():
    """[128, NP*HEADS] indicator: col p*8+h is 1 on rows of chunk p in head h."""
    ind = np.zeros((128, NP * HEADS), np.float32)
    for p in range(NP):
        lo, hi = 128 * p, min(128 * p + 128, KP)
        for h in range(HEADS):
            a, b = max(lo, SKEY * h), min(hi, SKEY * h + SKEY)
            if a < b:
                ind[a - lo : b - lo, p * HEADS + h] = 1.0
    return ind


def _bcind():
    """[HEADS, NP*128] indicator: row h is 1 on packed rows of head h (pad
    rows all-zero, so probs on pad rows normalize to exactly 0)."""
    ind = np.zeros((HEADS, NP * 128), np.float32)
    for h in range(HEADS):
        ind[h, SKEY * h : SKEY * (h + 1)] = 1.0
    return ind


def fold_weights(inputs):
    wq = _fold(inputs["Pq_in"], inputs["Wq"], inputs["Pq_out"], inputs["q_scale"])
    wk = _fold(inputs["Pk_in"], inputs["Wk"], inputs["Pk_out"], inputs["k_scale"])
    wv = _fold(inputs["Pv_in"], inputs["Wv"], inputs["Pv_out"], inputs["v_scale"])
    wo = _fold(inputs["Pout_in"], inputs["Wout"], inputs["Pout_out"],
               inputs["out_scale"])
    wq = wq[:, HEAD_PERM]
    wk = wk[:, HEAD_PERM]
    wv = wv[:, HEAD_PERM]
    wo = wo[HEAD_PERM, :]
    return (wq.astype(np.float32), wk.astype(np.float32),
            wv.astype(np.float32), wo.astype(np.float32))


def _pack_w(W):  # [K*128, M] -> [128, K*M]
    Kc = W.shape[0] // 128
    return np.ascontiguousarray(
        W.reshape(Kc, 128, W.shape[1]).transpose(1, 0, 2).reshape(128, -1))


def make_in_map(x_b, enc_b, wq, wk, wv, wo):
    xt = (x_b.T.reshape(KH, 128, NT, SQ).transpose(2, 1, 0, 3)
          .reshape(NT, 128, 2, XH).transpose(0, 2, 1, 3))
    xt = np.ascontiguousarray(xt).astype(NPBF16)     # [NT, 2, 128, XH]
    encp = np.zeros((SKP, CROSS), np.float32)
    encp[:SKEY] = enc_b
    enct = _pack_w(np.ascontiguousarray(encp.T)).astype(NPBF16)
    return {
        "xt": xt,
        "wqt": _pack_w(np.ascontiguousarray(wq.T)).astype(NPBF16),
        "wk": _pack_w(wk).astype(NPBF16),
        "wv": _pack_w(wv).astype(NPBF16),
        "wo": _pack_w(wo).astype(NPBF16),
        "enct": enct,
        "sumind": _sumind().astype(NPBF16),
        "bcind": _bcind().astype(NPBF16),
    }


def build_nc(loop_reps=1):
    nc = bacc.Bacc("TRN2", target_bir_lowering=False, debug=False)
    xt_d = nc.dram_tensor("xt", [NT, 2, 128, XH], BF16, kind="ExternalInput").ap()
    wqt_d = nc.dram_tensor("wqt", [128, KH * HID], BF16, kind="ExternalInput").ap()
    wk_d = nc.dram_tensor("wk", [128, KC * HID], BF16, kind="ExternalInput").ap()
    wv_d = nc.dram_tensor("wv", [128, KC * HID], BF16, kind="ExternalInput").ap()
    wo_d = nc.dram_tensor("wo", [128, KH * HID], BF16, kind="ExternalInput").ap()
    enct_d = nc.dram_tensor("enct", [128, KC * SKP], BF16, kind="ExternalInput").ap()
    si_d = nc.dram_tensor("sumind", [128, NP * HEADS], BF16,
                          kind="ExternalInput").ap()
    bi_d = nc.dram_tensor("bcind", [HEADS, NP * 128], BF16,
                          kind="ExternalInput").ap()
    out_d = nc.dram_tensor("out", [SEQ, HID], BF16, kind="ExternalOutput").ap()

    with tile.TileContext(nc) as tc:
        with ExitStack() as ctx:
            ctx.enter_context(nc.allow_low_precision(
                "bf16 matmul inputs and SBUF intermediates; f32 PSUM accum"))
            const = ctx.enter_context(tc.tile_pool(name="const", bufs=1))
            si_t = const.tile([128, NP * HEADS], BF16, name="si_t")
            nc.sync.dma_start(si_t[:], si_d)
            bi_t = const.tile([HEADS, NP * 128], BF16, name="bi_t")
            nc.sync.dma_start(bi_t[:], bi_d)
            wqt_t = const.tile([128, KH * HID], BF16, name="wqt_t")
            kt_t = const.tile([128, KH * SKP], BF16, name="kt_t")
            vt_t = const.tile([128, KH * SKP], BF16, name="vt_t")
            wo_t = const.tile([128, KH * HID], BF16, name="wo_t")
            mt_t = const.tile([128, KH * KPP], BF16, name="mt_t")
            nt_t = const.tile([128, NP * HID], BF16, name="nt_t")

            xt_pool = ctx.enter_context(tc.tile_pool(name="xt", bufs=4))
            exp_pool = ctx.enter_context(tc.tile_pool(name="exp", bufs=10))
            rc_pool = ctx.enter_context(tc.tile_pool(name="rc", bufs=2))
            out_pool = ctx.enter_context(tc.tile_pool(name="outsb", bufs=2))

            if loop_reps > 1:
                # hint_engines: prefetch the back-edge target so the ~1700-inst
                # body doesn't eat an I$ miss per iteration
                ctx.enter_context(tc.For_i(
                    0, loop_reps, 1,
                    hint_engines=(mybir.EngineType.PE, mybir.EngineType.DVE,
                                  mybir.EngineType.Activation,
                                  mybir.EngineType.SP, mybir.EngineType.Pool)))

            ev_ctr = [0]

            def ev_copy(dst, src):
                """PSUM->SBUF eviction, alternating Act / DVE."""
                ev_ctr[0] += 1
                if ev_ctr[0] % 2:
                    nc.scalar.activation(dst, src,
                                         mybir.ActivationFunctionType.Copy)
                else:
                    nc.vector.tensor_copy(dst, src)

            xt_tiles = {}

            def xt_dma(t):
                xh = []
                for hf in range(2):
                    xx = xt_pool.tile([128, XH], BF16, tag="xt",
                                      name=f"xt{t}_{hf}")
                    nc.sync.dma_start(xx[:], xt_d[t, hf])
                    xh.append(xx)
                xt_tiles[t] = xh

            # zero M's packed-key pad columns and all of N (evictions then fill
            # N's real rows; pad rows must be 0.0 -- probs there are exactly 0
            # but 0*NaN from stale SBUF would poison the PSUM accumulation)
            for c in range(KH):
                nc.gpsimd.memset(mt_t[:, c * KPP + KP : c * KPP + KPP], 0.0)
            nc.gpsimd.memset(nt_t[:], 0.0)

            # ------- setup: KT / VT, then M = WqT@KT-cols, N = V@Wout
            with tc.tile_pool(name="setup_e", bufs=1) as setup_e, \
                 tc.tile_pool(name="setup_w", bufs=1) as setup_w:
                # DMA issue order = need order: enct/wk (KT), wv (VT),
                # wqt (M), wo (N), then xt(0) was already queued above
                enct_t = setup_e.tile([128, KC * SKP], BF16, name="enct_t")
                nc.sync.dma_start(enct_t[:], enct_d)
                wk_t = setup_w.tile([128, KC * HID], BF16, name="wk_t")
                nc.sync.dma_start(wk_t[:], wk_d)
                wv_t = setup_w.tile([128, KC * HID], BF16, name="wv_t")
                nc.sync.dma_start(wv_t[:], wv_d)
                nc.sync.dma_start(wqt_t[:], wqt_d)
                nc.sync.dma_start(wo_t[:], wo_d)
                xt_dma(0)
                with tc.tile_pool(name="psum_setup", bufs=4,
                                  space="PSUM") as psum_s:
                    for wi, (wt, dst) in enumerate(((wk_t, kt_t), (wv_t, vt_t))):
                        for m in range(KH):
                            pk = psum_s.tile([128, SKP], F32, tag="pk",
                                             name=f"pk{wi}_{m}")
                            for k in range(KC):
                                nc.tensor.matmul(
                                    pk[:],
                                    wt[:, k * HID + m * 128 : k * HID + (m + 1) * 128],
                                    enct_t[:, k * SKP : (k + 1) * SKP],
                                    start=(k == 0), stop=(k == KC - 1),
                                )
                            ev_copy(dst[:, m * SKP : (m + 1) * SKP], pk[:])

                # M / N precompute, interleaved so PSUM-eviction and DMA
                # latencies hide behind each other's matmuls.
                # M: 4 heads share one [128, 308] PSUM tile per f-chunk m.
                # N: per head, 3 n-tiles evict into one [77, HID] scratch,
                #    then 1-2 partition-offset DMAs into nt's packed rows
                #    (engine APs can only start at partition 0/32/64/96;
                #    DMAs are exempt).
                with tc.tile_pool(name="pmt", bufs=3, space="PSUM") as pmt_p, \
                     tc.tile_pool(name="pnt", bufs=3, space="PSUM") as pnt_p, \
                     tc.tile_pool(name="nscr", bufs=3) as nscr:

                    def mt_chunk(hg, m):
                        pm = pmt_p.tile([128, 4 * SKEY], F32, tag="pm",
                                        name=f"pm{hg}_{m}")
                        for hi in range(4):
                            h = 4 * hg + hi
                            for i, (c, o, L) in enumerate(_head_pieces(h)):
                                nc.tensor.matmul(
                                    pm[:, SKEY * hi : SKEY * (hi + 1)],
                                    wqt_t[o : o + L, c * HID + m * 128 : c * HID + (m + 1) * 128],
                                    kt_t[o : o + L, c * SKP : c * SKP + SKEY],
                                    start=(i == 0), stop=(i == 1),
                                    tile_position=(o, 0),
                                )
                        ev_copy(mt_t[:, m * KPP + 4 * SKEY * hg :
                                     m * KPP + 4 * SKEY * (hg + 1)], pm[:])

                    def n_chunk(h):
                        ns = nscr.tile([SKEY, HID], BF16, tag="ns", name=f"ns{h}")
                        for (n_off, n_sz) in NTILES:
                            pn = pnt_p.tile([SKEY, n_sz], F32, tag="pn",
                                            name=f"pn{h}_{n_off}")
                            for i, (c, o, L) in enumerate(_head_pieces(h)):
                                nc.tensor.matmul(
                                    pn[:],
                                    vt_t[o : o + L, c * SKP : c * SKP + SKEY],
                                    wo_t[o : o + L, c * HID + n_off : c * HID + n_off + n_sz],
                                    start=(i == 0), stop=(i == 1),
                                    tile_position=(o, 0),
                                )
                            ev_copy(ns[:, n_off : n_off + n_sz], pn[:])
                        r0 = SKEY * h
                        while r0 < SKEY * (h + 1):
                            p = r0 // 128
                            r1 = min(SKEY * (h + 1), 128 * (p + 1))
                            nc.sync.dma_start(
                                nt_t[r0 - 128 * p : r1 - 128 * p, p * HID : (p + 1) * HID],
                                ns[r0 - SKEY * h : r1 - SKEY * h, :])
                            r0 = r1

                    for hg in range(2):
                        for m in range(KH):
                            mt_chunk(hg, m)
                            for h4 in range(4):
                                if m == 2 + 2 * h4:
                                    n_chunk(4 * hg + h4)

            psc = ctx.enter_context(tc.tile_pool(name="psc", bufs=2, space="PSUM"))
            psm = ctx.enter_context(tc.tile_pool(name="psm", bufs=2, space="PSUM"))
            pmm = ctx.enter_context(tc.tile_pool(name="pmm", bufs=2, space="PSUM"))
            pbc = ctx.enter_context(tc.tile_pool(name="pbc", bufs=2, space="PSUM"))

            exp_sets = {}

            def g_work(t):
                """out-rows of tile t: 12 (j, n)-groups x 5 packed-chunk
                accumulate, split into 3+2-matmul chunks for gap filling."""
                exps = exp_sets.pop(t)
                sbs = {}
                state = {}

                def mk(j, n_off, n_sz, p0, pn_, last):
                    def run():
                        if j not in sbs:
                            sbs[j] = out_pool.tile([128, HID], BF16, tag="osb",
                                                   name=f"ob{t}_{j}")
                        if p0 == 0:
                            state["po"] = pmm.tile([128, n_sz], F32, tag="mm",
                                                   name=f"po{t}_{j}_{n_off}")
                        po = state["po"]
                        for p in range(p0, p0 + pn_):
                            nc.tensor.matmul(
                                po[:],
                                exps[p][:, j * 128 : (j + 1) * 128],
                                nt_t[:, p * HID + n_off : p * HID + n_off + n_sz],
                                start=(p == 0), stop=(p == NP - 1),
                            )
                        if last:
                            ev_copy(sbs[j][:, n_off : n_off + n_sz], po[:])
                            if n_off == NTILES[-1][0]:
                                nc.sync.dma_start(
                                    out_d[t * SQ + j * 128 : t * SQ + (j + 1) * 128, :],
                                    sbs[j][:],
                                )
                    return run

                chunks = []
                for j in range(SQ // 128):
                    for (n_off, n_sz) in NTILES:
                        chunks.append(mk(j, n_off, n_sz, 0, 3, False))
                        chunks.append(mk(j, n_off, n_sz, 3, NP - 3, True))
                return chunks

            def phase_front(t, gchunks):
                """scores+exp+sums, then normalize; G(t-1) chunks paced in."""
                if t + 1 < NT:
                    xt_dma(t + 1)
                xh = xt_tiles.pop(t)
                n_points = 3 * NP + NP
                quota = len(gchunks) / n_points
                acc = [0.0]

                def gfill():
                    acc[0] += quota
                    while acc[0] >= 1.0 and gchunks:
                        gchunks.pop(0)()
                        acc[0] -= 1.0

                exps = []
                sm = psm.tile([HEADS, SQ], F32, tag="sm", name=f"sm{t}")
                for p in range(NP):
                    sc = psc.tile([128, SQ], F32, tag="sc", name=f"sc{t}_{p}")
                    for c in range(KH):
                        nc.tensor.matmul(
                            sc[:],
                            mt_t[:, c * KPP + 128 * p : c * KPP + 128 * (p + 1)],
                            xh[c // 5][:, (c % 5) * SQ : (c % 5 + 1) * SQ],
                            start=(c == 0), stop=(c == KH - 1),
                        )
                        if c == 4:
                            gfill()
                    e = exp_pool.tile([128, SQ], BF16, tag="exp",
                                      name=f"ex{t}_{p}")
                    nc.scalar.activation(
                        e[:], sc[:],
                        mybir.ActivationFunctionType.Exp, scale=ATTN_SCALE,
                    )
                    gfill()
                    nc.tensor.matmul(sm[:], si_t[:, p * HEADS : (p + 1) * HEADS],
                                     e[:], start=(p == 0), stop=(p == NP - 1))
                    exps.append(e)
                    gfill()
                rc = rc_pool.tile([HEADS, SQ], BF16, tag="rc", name=f"rc{t}")
                nc.vector.reciprocal(rc[:], sm[:])
                for p in range(NP):
                    # bc[r, q] = rc[head(r), q] via indicator matmul (PE); pad
                    # rows get 0, so probs there are exactly 0
                    bc = pbc.tile([128, SQ], F32, tag="bc", name=f"bc{t}_{p}")
                    nc.tensor.matmul(bc[:], bi_t[:, p * 128 : (p + 1) * 128],
                                     rc[:], start=True, stop=True)
                    nc.vector.tensor_tensor(exps[p][:], exps[p][:], bc[:],
                                            mybir.AluOpType.mult)
                    gfill()
                while gchunks:
                    gchunks.pop(0)()
                exp_sets[t] = exps

            for t in range(NT):
                phase_front(t, g_work(t - 1) if t > 0 else [])
            for run in g_work(NT - 1):
                run()

    nc.finalize()
    return nc


from concourse.bass_utils import run_bass_kernel_spmd

_NC_CACHE = {}


def _get_nc(loop_reps=1):
    if loop_reps not in _NC_CACHE:
        _NC_CACHE[loop_reps] = build_nc(loop_reps)
    return _NC_CACHE[loop_reps]


def kernel(**inputs):
    inputs = {k: np.asarray(v) for k, v in inputs.items()}
    wq, wk, wv, wo = fold_weights(inputs)
    x = inputs["hidden_states"].astype(np.float32, copy=False)
    enc = inputs["encoder_hidden_states"].astype(np.float32, copy=False)
    B = x.shape[0]
    in_maps = [make_in_map(x[b], enc[b], wq, wk, wv, wo) for b in range(B)]
    nc = _get_nc()
    res = run_bass_kernel_spmd(nc, in_maps, list(range(B)))
    bout = inputs["bout"].astype(np.float32, copy=False)
    return np.stack([
        np.asarray(res.results[b]["out"]).astype(np.float32) + bout[None, :]
        for b in range(B)
    ])
